# revision 20
# baseline (speedup 1.0000x reference)
"""Trainium2 Bass kernel for nn_AttentionBlock (GroupNorm -> MHA -> proj -> residual).

Shapes (hardcoded): x [16, 512, 32, 32] fp32, NUM_GROUPS=32, NUM_HEADS=8.
Sharding: data-parallel over batch: 8 cores x 2 batches each. No collectives.

Per-core algorithm (B_loc=2, C=512, S=1024, nh=8, hd=64), fp8 DoubleRow:
  All matmul stages run in fp8e4 with MatmulPerfMode.DoubleRow ([K,2,M] x
  [K,2,N] -> [M,N] at 0.5 PE cycles/row): QKV/V/AV/proj pack 2 real k-tiles
  per instruction (4x cheaper than bf16); QK packs its 64-wide contraction
  as a stride-0 broadcast j-pair (logits doubled, q pre-scale halved to
  1/(2*sqrt(hd))), 2x cheaper. ACT exp (~128us) is the roofline.
  GroupNorm: fp32 bn_stats per channel; cross-partition group aggregation
    via block-diagonal f32r matmul; rsqrt as exp(-0.5*ln(var+eps)) so the
    whole kernel uses ONE activation table (no LoadActFuncSet switches);
    h = x*s + b' stored fp8e4 in paired c-tiles h2[cp] = [128, (j, s)].
    Batch-0 h is written by ACT Identity (idle pre-exp); batch-1 on DVE.
    Batch-1 GN is emitted inside batch-0's head-0 exp window so its DVE
    work never delays batch-0's q/k epilogues.
  QKV: q,k in [o, s] o-tile layout (head pair per tile at partition 0/64);
    q bias folded, k bias dropped (softmax-invariant).
    v^T per t-tile pair: v2[m] = [t128, (j, head, 66)], ones col at 64.
  Attention: logits^T[t,s] per (head,tt,sc) via one DoubleRow matmul;
    exp (no max subtraction: |logit| < ~3) -> fp8 P^T pairs pt2[m] =
    [t128, (j, s)]; AV accumulates 4 DoubleRow matmuls (M=65, ones row ->
    row 64 = softmax denominator Z); normalize via DVE reciprocal + gpsimd
    partition_broadcast + DVE multiply -> fp8 ao2[cp] = [128, (j, s)].
  Proj: DoubleRow over 2 c-pairs; epilogue is one scalar_tensor_tensor:
    out = (proj_psum + b_eff) + x, with x read from the resident xg tiles
    (b_eff = proj_b + proj_w @ b_v; valid because sum_t P/Z = 1 exactly).

Engine budget per core (modeled): ACT ~130us (exp-bound), DVE ~85us,
PE ~58us, POOL ~26us.
"""
import numpy as np
import ml_dtypes

import concourse.bass as bass
import concourse.bacc as bacc
import concourse.tile as tile
import concourse.mybir as mybir
from concourse import bass_utils

F32 = mybir.dt.float32
F32R = mybir.dt.float32r
F8 = mybir.dt.float8e4
I8 = mybir.dt.int8
AF = mybir.ActivationFunctionType
ALU = mybir.AluOpType
DR = mybir.MatmulPerfMode.DoubleRow

NCORES = 8
B, C, H, W = 16, 512, 32, 32
S = H * W                 # 1024
NH, HD = 8, 64
G, GS = 32, 16            # groups, channels per group
BLOC = B // NCORES        # 2
CT = C // 128             # 4 channel tiles
CP = CT // 2              # 2 channel-tile pairs
TT = S // 128             # 8 t tiles
TM = TT // 2              # 4 t-tile pairs
SC = S // 512             # 2 s chunks of 512
EPS = 1e-5

_NC_CACHE = {}


def f32r_round(x: np.ndarray) -> np.ndarray:
    """fp32 -> fp32r: round mantissa to 11 bits (round-half-away on bit 12)."""
    u = np.ascontiguousarray(x, dtype=np.float32).view(np.uint32)
    lsb = (u >> 12) & np.uint32(1)
    r = u + np.uint32(0x7FF) + lsb
    return (r & np.uint32(0xFFFFF000)).view(np.float32)


def build_program():
    # The greedy act-table picker takes the FIRST set containing each
    # function: Ln would pick the ln-only 'natural_log' set and every
    # following Exp would reload a different one (1.28us per reload, some
    # mid exp-stream). Blank 'natural_log' in the cached dict (shared
    # object, index positions preserved) so Ln resolves to
    # 'natural_log_exp_and_others' and the whole kernel runs off one table.
    if not getattr(bacc, "_nlx_patched", False):
        _orig_gat = bacc.get_activation_tables

        def _patched_gat(arch):
            tabs = dict(_orig_gat(arch))
            if "natural_log" in tabs:
                tabs["natural_log"] = set()
            return tabs

        bacc.get_activation_tables = _patched_gat
        bacc._nlx_patched = True

    nc = bacc.Bacc("TRN2", target_bir_lowering=False, debug=False)

    x_d = nc.dram_tensor("x", [BLOC, C, S], F32, kind="ExternalInput").ap()
    # [c_local, (cp 2, j 2, o 1024)]; o: 512 q channels (pre-scaled), 512 k
    wqk_d = nc.dram_tensor("wqk", [128, 2 * 2 * 1024], F8,
                           kind="ExternalInput").ap()
    # [c_local, (cp 2, j 2, o 512)]
    wv_d = nc.dram_tensor("wv", [128, 2 * 2 * 512], F8, kind="ExternalInput").ap()
    wp_d = nc.dram_tensor("wp", [128, 2 * 2 * 512], F8, kind="ExternalInput").ap()
    # packed small consts: [bq 4 | beff 4 | gamma 4 | beta 4 | gmat 128]
    smalls_d = nc.dram_tensor("smalls", [128, 144], F32R,
                              kind="ExternalInput").ap()
    out_d = nc.dram_tensor("out", [BLOC, C, S], F32, kind="ExternalOutput").ap()

    with tile.TileContext(nc) as tc:
        import contextlib
        with contextlib.ExitStack() as ctx:
            consts = ctx.enter_context(tc.tile_pool(name="consts", bufs=1))
            xg_pool = ctx.enter_context(tc.tile_pool(name="xg", bufs=2 * CT))
            small = ctx.enter_context(tc.tile_pool(name="small", bufs=4))
            h_pool = ctx.enter_context(tc.tile_pool(name="h", bufs=2 * CP))
            qk_pool = ctx.enter_context(tc.tile_pool(name="qk", bufs=16))
            v_pool = ctx.enter_context(tc.tile_pool(name="v", bufs=2 * TM))
            pt_pool = ctx.enter_context(tc.tile_pool(name="pt", bufs=4 * TM))
            ao_pool = ctx.enter_context(tc.tile_pool(name="ao", bufs=2 * CP))
            z_pool = ctx.enter_context(tc.tile_pool(name="z", bufs=2))
            res_pool = ctx.enter_context(tc.tile_pool(name="res", bufs=8))
            ps = ctx.enter_context(tc.tile_pool(name="ps", bufs=2, space="PSUM"))

            # ---- DMA order (one serial pipe, ~1.46us per x tile): small
            # gn constants, x b0, x b1, then weights (first QKV needs wqk
            # only at ~15us). xg tiles stay resident until the proj
            # residual reads them ----
            smalls_sb = consts.tile([128, 144], F32R, name="smalls")
            nc.sync.dma_start(out=smalls_sb, in_=smalls_d)
            bq_sb = smalls_sb[:, 0:4].bitcast(F32)
            beff_sb = smalls_sb[:, 4:8].bitcast(F32)
            gamma_sb = smalls_sb[:, 8:12].bitcast(F32)
            beta_sb = smalls_sb[:, 12:16].bitcast(F32)
            gmat_sb = smalls_sb[:, 16:144]
            eps_sb = consts.tile([128, 1], F32)
            nc.vector.memset(eps_sb, EPS)
            xg_tiles = {}
            for ci in range(CT):
                xg = xg_pool.tile([128, S], F32, tag="xg", name=f"xg0_{ci}")
                nc.sync.dma_start(out=xg, in_=x_d[0, ci * 128:(ci + 1) * 128, :])
                xg_tiles[(0, ci)] = xg
            wqk_sb = consts.tile([128, 2, 2, 1024], F8, name="wqk")
            nc.sync.dma_start(out=wqk_sb, in_=wqk_d)
            for ci in range(CT):
                xg = xg_pool.tile([128, S], F32, tag="xg", name=f"xg1_{ci}")
                nc.sync.dma_start(out=xg, in_=x_d[1, ci * 128:(ci + 1) * 128, :])
                xg_tiles[(1, ci)] = xg
            wv_sb = consts.tile([128, 2, 2, 512], F8, name="wv")
            nc.sync.dma_start(out=wv_sb, in_=wv_d)
            wp_sb = consts.tile([128, 2, 2, 512], F8, name="wp")
            nc.sync.dma_start(out=wp_sb, in_=wp_d)

            # ---- groupnorm; h stored fp8 in paired c-tiles ----
            h_tiles = {}   # (b, cp) -> [128, 2, S] fp8
            for b in range(BLOC):
                for cp in range(CP):
                    h_tiles[(b, cp)] = h_pool.tile(
                        [128, 2, S], F8, tag="h", name=f"h{b}_{cp}")

            def emit_gn(b):
                # per-ci bn stats/aggr, then ONE batched small-op chain over
                # all 4 ci columns (short DVE critical chain at startup)
                stats4 = small.tile([128, CT, 2, 6], F32, tag="stats4")
                mv4 = small.tile([128, CT, 2], F32, tag="mv4")
                for ci in range(CT):
                    xg2 = xg_tiles[(b, ci)].rearrange("p (n f) -> p n f", f=512)
                    nc.vector.bn_stats(out=stats4[:, ci, 0, :], in_=xg2[:, 0, :])
                    nc.vector.bn_stats(out=stats4[:, ci, 1, :], in_=xg2[:, 1, :])
                for ci in range(CT):
                    nc.vector.bn_aggr(out=mv4[:, ci, :], in_=stats4[:, ci, :, :])
                sin4 = small.tile([128, CT, 2], F32R, tag="sin4")
                nc.vector.tensor_copy(sin4[:, :, 0:1], mv4[:, :, 0:1])
                nc.vector.tensor_tensor(
                    out=sin4[:, :, 1:2], in0=mv4[:, :, 0:1], in1=mv4[:, :, 0:1],
                    op=ALU.mult)
                nc.vector.tensor_tensor(
                    out=sin4[:, :, 1:2], in0=sin4[:, :, 1:2].bitcast(F32),
                    in1=mv4[:, :, 1:2], op=ALU.add)
                gp4 = ps.tile([128, 2 * CT], F32, tag="mm")
                nc.tensor.matmul(gp4, gmat_sb,
                                 sin4.rearrange("p a b -> p (a b)"),
                                 start=True, stop=True)
                gg4 = small.tile([128, CT, 2], F32, tag="gg4")
                nc.vector.tensor_copy(gg4.rearrange("p a b -> p (a b)"), gp4)
                gv4 = small.tile([128, 5, CT], F32, tag="gv4")
                mean4 = gg4[:, :, 0:1].rearrange("p a b -> p (a b)")
                ex24 = gg4[:, :, 1:2].rearrange("p a b -> p (a b)")
                nc.vector.tensor_tensor(
                    out=gv4[:, 0, :], in0=mean4, in1=mean4, op=ALU.mult)
                nc.vector.tensor_tensor(
                    out=gv4[:, 0, :], in0=ex24, in1=gv4[:, 0, :], op=ALU.subtract)
                # rsqrt(var+eps) = exp(-0.5*ln(var+eps)): stays in the Exp
                # activation table (a Sqrt would force table reloads)
                nc.scalar.activation(gv4[:, 4, :], gv4[:, 0, :], AF.Ln,
                                     bias=eps_sb)
                nc.scalar.activation(gv4[:, 1, :], gv4[:, 4, :], AF.Exp,
                                     scale=-0.5)
                # tiny post-exp muls on Pool: keeps them out of the DVE
                # stream, which the tile scheduler may reorder behind
                # DMA-blocked bn_stats of the other batch
                nc.gpsimd.tensor_tensor(
                    out=gv4[:, 2, :], in0=gv4[:, 1, :], in1=gamma_sb[:, 0:CT],
                    op=ALU.mult)
                nc.gpsimd.tensor_tensor(
                    out=gv4[:, 3, :], in0=mean4, in1=gv4[:, 2, :], op=ALU.mult)
                nc.gpsimd.tensor_tensor(
                    out=gv4[:, 3, :], in0=beta_sb[:, 0:CT], in1=gv4[:, 3, :],
                    op=ALU.subtract)
                for ci in range(CT):
                    hdst = h_tiles[(b, ci // 2)][:, ci % 2, :]
                    if b == 0 and ci < 2:
                        # ACT is idle before the first exp; split h-writes
                        # ACT/DVE so neither engine serializes all four
                        nc.scalar.activation(
                            hdst, xg_tiles[(b, ci)], AF.Identity,
                            bias=gv4[:, 3, ci:ci + 1], scale=gv4[:, 2, ci:ci + 1])
                    else:
                        nc.vector.tensor_scalar(
                            out=hdst, in0=xg_tiles[(b, ci)],
                            scalar1=gv4[:, 2, ci:ci + 1],
                            scalar2=gv4[:, 3, ci:ci + 1],
                            op0=ALU.mult, op1=ALU.add)

            emit_gn(0)

            # ---- per batch: QKV -> attention -> proj ----
            qk_tiles = {}  # (b, ot) -> [128, S] fp8; ot 0-3 q, 4-7 k
            v_tiles = {}   # (b, m) -> [128, 2, NH, 66] fp8

            def emit_qk_tile(b, ot, k_on_act=False):
                qt = qk_pool.tile([128, S], F8, tag="qk", name=f"qk{b}_{ot}")
                for sc in range(SC):
                    pp = ps.tile([128, 512], F32, tag="mm", name="qkmm")
                    for cp in range(CP):
                        nc.tensor.matmul(
                            pp,
                            wqk_sb[:, cp, :, ot * 128:(ot + 1) * 128],
                            h_tiles[(b, cp)][:, :, sc * 512:(sc + 1) * 512],
                            start=(cp == 0), stop=(cp == CP - 1),
                            perf_mode=DR)
                    dst = qt[:, sc * 512:(sc + 1) * 512]
                    if ot < 4:
                        nc.vector.tensor_scalar(
                            out=dst, in0=pp, scalar1=bq_sb[:, ot:ot + 1],
                            scalar2=None, op0=ALU.add)
                    elif k_on_act:
                        # pre-exp ACT is idle: parallelize the first k
                        # epilogues with the q ones on DVE
                        nc.scalar.activation(dst, pp, AF.Identity)
                    else:
                        nc.vector.tensor_copy(dst, pp)
                qk_tiles[(b, ot)] = qt

            def emit_v_tiles(b):
                for m in range(TM):
                    vt = v_pool.tile([128, 2, NH, 66], F8, tag="v",
                                     name=f"v{b}_{m}")
                    for jt in range(2):         # t tile within pair
                        st = 2 * m + jt
                        pp = ps.tile([128, 512], F32, tag="mm", name="vmm")
                        for cp in range(CP):
                            nc.tensor.matmul(
                                pp,
                                h_tiles[(b, cp)][:, :, st * 128:(st + 1) * 128],
                                wv_sb[:, cp, :, :],
                                start=(cp == 0), stop=(cp == CP - 1),
                                perf_mode=DR)
                        nc.vector.tensor_copy(
                            vt[:, jt, :, 0:64],
                            pp.rearrange("p (h e) -> p h e", e=64))
                    nc.vector.memset(vt[:, :, :, 64:65], 1.0)
                    v_tiles[(b, m)] = vt

            ao_all = {}

            def emit_proj(b):
                for ot in range(CT):
                    for sc in range(SC):
                        pp = ps.tile([128, 512], F32, tag="mm")
                        for cp in range(CP):
                            nc.tensor.matmul(
                                pp,
                                wp_sb[:, cp, :, ot * 128:(ot + 1) * 128],
                                ao_all[b][cp][:, :, sc * 512:(sc + 1) * 512],
                                start=(cp == 0), stop=(cp == CP - 1),
                                perf_mode=DR)
                        ro = res_pool.tile([128, 512], F32, tag="ro")
                        nc.vector.scalar_tensor_tensor(
                            out=ro, in0=pp, scalar=beff_sb[:, ot:ot + 1],
                            in1=xg_tiles[(b, ot)][:, sc * 512:(sc + 1) * 512],
                            op0=ALU.add, op1=ALU.add)
                        # tail outputs split across the SP and ACT queues
                        eng = nc.sync if (b == 0 or (ot * SC + sc) % 2 == 0) \
                            else nc.scalar
                        eng.dma_start(
                            out=out_d[b, ot * 128:(ot + 1) * 128,
                                      sc * 512:(sc + 1) * 512],
                            in_=ro)

            # exp load-balancing: DVE computes some tiles with a one-op
            # Schraudolph exp directly in fp8e4m3 bits:
            #   int8 = round(logit * 8/ln2 + SCH_B)  ==  bits of exp(logit)
            # (piecewise-linear 2^x; error ~ fp8 quantization, verified
            # numerically to match the table-exp path's accuracy)
            SCH_A = 8.0 / float(np.log(2.0))
            SCH_B = 55.75
            DVE_EXP = {(0, 0): 0, (0, 1): 0, (0, 2): 1, (0, 3): 1, (1, 0): 1}
            DVE_TILES = {0: set(), 1: {(0, 1)}, 2: {(0, 1), (2, 1)},
                         3: {(0, 1), (1, 1), (2, 1)}}

            def emit_qk_exp(b, h):
                """logits + exp for head h -> pt tiles [128, 2, S] fp8."""
                p0 = 64 * (h % 2)
                kt = qk_tiles[(b, 4 + h // 2)]
                qt = qk_tiles[(b, h // 2)]
                dve_tiles = DVE_TILES[DVE_EXP.get((b, h), 2)]
                pts = []
                for m in range(TM):
                    pt = pt_pool.tile([128, 2, S], F8, tag="pt",
                                      name=f"pt{b}_{h}_{m}")
                    for jt in range(2):
                        t0 = (2 * m + jt) * 128
                        lg = ps.tile([128, 1024], F32, tag="qk", name="lg")
                        for sc in range(SC):
                            nc.tensor.matmul(
                                lg[:, sc * 512:(sc + 1) * 512],
                                kt[p0:p0 + 64, t0:t0 + 128]
                                .unsqueeze(1).broadcast_to([64, 2, 128]),
                                qt[p0:p0 + 64, sc * 512:(sc + 1) * 512]
                                .unsqueeze(1).broadcast_to([64, 2, 512]),
                                start=True, stop=True, perf_mode=DR)
                        if (m, jt) in dve_tiles:
                            nc.vector.tensor_scalar(
                                out=pt[:, jt, :].bitcast(I8), in0=lg,
                                scalar1=SCH_A, scalar2=SCH_B,
                                op0=ALU.mult, op1=ALU.add)
                        else:
                            nc.scalar.activation(pt[:, jt, :], lg, AF.Exp)
                    pts.append(pt)
                return pts

            def emit_av(b, h, pts):
                cp, j, r0 = h // 4, (h // 2) % 2, (h % 2) * 64
                for sc in range(SC):
                    av = ps.tile([65, 512], F32, tag="av")
                    for m in range(TM):
                        nc.tensor.matmul(
                            av,
                            v_tiles[(b, m)][:, :, h, 0:65],
                            pts[m][:, :, sc * 512:(sc + 1) * 512],
                            start=(m == 0), stop=(m == TM - 1),
                            perf_mode=DR)
                    zr = z_pool.tile([1, 512], F32, tag="zr")
                    nc.vector.reciprocal(out=zr, in_=av[64:65, :])
                    zb = z_pool.tile([64, 512], F32, tag="zb")
                    nc.gpsimd.partition_broadcast(zb, zr, channels=64)
                    nc.vector.tensor_tensor(
                        out=ao_all[b][cp][r0:r0 + 64, j,
                                          sc * 512:(sc + 1) * 512],
                        in0=av[0:64, :], in1=zb, op=ALU.mult)

            for b in range(BLOC):
                for cp in range(CP):
                    ao_all.setdefault(b, {})[cp] = ao_pool.tile(
                        [128, 2, S], F8, tag="ao", name=f"ao{b}_{cp}")

            # PE warm-up: keep PE continuously busy through the DMA/GN
            # startup so the first QKV matmuls run at full clock (any PE
            # idle resets the pstate ramp)

            emit_qk_tile(0, 0, k_on_act=True)
            emit_qk_tile(0, 4, k_on_act=True)
            emit_v_tiles(0)
            # (b1 GN + prep are hoisted into b0's exp windows below)

            prev = None
            for b, h in [(bb, hh) for bb in range(BLOC) for hh in range(NH)]:
                cur = emit_qk_exp(b, h)
                if b == 0 and h == 0:
                    # nearest exp deadline first: b0's next pair
                    emit_qk_tile(0, 1)
                    emit_qk_tile(0, 5)
                if b == 0 and h == 1:
                    # b1 GN here (not h0): its gmat matmul waits on b1's
                    # DVE stats; at h0 that stall would block QK(h1) on
                    # the in-order PE queue
                    emit_qk_tile(0, 2)
                    emit_qk_tile(0, 6)
                    emit_gn(1)
                if b == 0 and h == 2:
                    emit_qk_tile(0, 3)
                    emit_qk_tile(0, 7)
                    emit_qk_tile(1, 0)
                    emit_qk_tile(1, 4)
                    emit_v_tiles(1)
                if b == 0 and h == 3:
                    for pr in range(1, 4):
                        emit_qk_tile(1, pr)
                        emit_qk_tile(1, pr + 4)
                if prev is not None:
                    emit_av(*prev)
                if b == 1 and h == 0:
                    # b0's proj here: after AV(b0,h7) wrote the last ao rows,
                    # and after QK(b1,h0) so b1's first exps aren't delayed
                    emit_proj(0)
                prev = (b, h, cur)
            emit_av(*prev)

            emit_proj(1)

    nc.compile()
    return nc


def prep_weights(norm_w, norm_b, qkv_w, qkv_b, proj_w, proj_b):
    """Host-side constant preprocessing."""
    f8 = ml_dtypes.float8_e4m3
    # extra 1/2: the QK DoubleRow broadcast pair sums the same product twice
    scale = 1.0 / (2.0 * np.sqrt(HD))
    wq = qkv_w[0:C] * scale
    wk = qkv_w[C:2 * C]
    wv = qkv_w[2 * C:3 * C]
    bq = qkv_b[0:C] * scale
    bv = qkv_b[2 * C:3 * C]

    # wqk2: [c_local, cp, j, o(1024)]; o 0-511 = q channels, 512-1023 = k
    wqk_cat = np.concatenate([wq, wk], axis=0)       # [1024, 512]
    wqk2 = np.zeros((128, 2, 2, 1024), dtype=f8)
    wv2 = np.zeros((128, 2, 2, 512), dtype=f8)
    wp2 = np.zeros((128, 2, 2, 512), dtype=f8)
    for cp in range(CP):
        for j in range(2):
            c0 = (2 * cp + j) * 128
            wqk2[:, cp, j, :] = wqk_cat[:, c0:c0 + 128].T.astype(f8)
            wv2[:, cp, j, :] = wv[:, c0:c0 + 128].T.astype(f8)
            wp2[:, cp, j, :] = proj_w[:, c0:c0 + 128].T.astype(f8)

    bq2 = bq.reshape(CT, 128).T.astype(np.float32)
    beff = (proj_b + proj_w @ bv).reshape(CT, 128).T.astype(np.float32)
    gamma = norm_w.reshape(CT, 128).T.astype(np.float32)
    beta = norm_b.reshape(CT, 128).T.astype(np.float32)
    gmat = np.zeros((128, 128), dtype=np.float32)
    for g in range(128 // GS):
        gmat[g * GS:(g + 1) * GS, g * GS:(g + 1) * GS] = 1.0 / GS
    gmat = f32r_round(gmat)
    smalls = np.concatenate([bq2, beff, gamma, beta, gmat], axis=1)
    return dict(wqk=wqk2.reshape(128, -1), wv=wv2.reshape(128, -1),
                wp=wp2.reshape(128, -1),
                smalls=np.ascontiguousarray(smalls.astype(np.float32)))


def kernel(x, norm_w, norm_b, qkv_w, qkv_b, proj_w, proj_b, _trace=False):
    x = np.ascontiguousarray(np.asarray(x, dtype=np.float32))
    consts = prep_weights(
        np.asarray(norm_w, np.float32), np.asarray(norm_b, np.float32),
        np.asarray(qkv_w, np.float32), np.asarray(qkv_b, np.float32),
        np.asarray(proj_w, np.float32), np.asarray(proj_b, np.float32))

    if "nc" not in _NC_CACHE:
        _NC_CACHE["nc"] = build_program()
    nc = _NC_CACHE["nc"]

    xr = x.reshape(B, C, S)
    in_maps = []
    for core in range(NCORES):
        m = dict(consts)
        m["x"] = np.ascontiguousarray(xr[core * BLOC:(core + 1) * BLOC])
        in_maps.append(m)

    res = bass_utils.run_bass_kernel_spmd(
        nc, in_maps, core_ids=list(range(NCORES)), trace=False)

    out = np.empty((B, C, S), dtype=np.float32)
    for core in range(NCORES):
        out[core * BLOC:(core + 1) * BLOC] = res.results[core]["out"]
    kernel.last_results = res
    return out.reshape(B, C, H, W)


# revision 21
# speedup vs baseline: 1.0144x; 1.0144x over previous
"""Trainium2 Bass kernel for nn_AttentionBlock (GroupNorm -> MHA -> proj -> residual).

Shapes (hardcoded): x [16, 512, 32, 32] fp32, NUM_GROUPS=32, NUM_HEADS=8.
Sharding: data-parallel over batch: 8 cores x 2 batches each. No collectives.

Per-core algorithm (B_loc=2, C=512, S=1024, nh=8, hd=64), fp8 DoubleRow:
  All matmul stages run in fp8e4 with MatmulPerfMode.DoubleRow ([K,2,M] x
  [K,2,N] -> [M,N] at 0.5 PE cycles/row): QKV/V/AV/proj pack 2 real k-tiles
  per instruction (4x cheaper than bf16); QK packs its 64-wide contraction
  as a stride-0 broadcast j-pair (logits doubled, q pre-scale halved to
  1/(2*sqrt(hd))), 2x cheaper. ACT exp (~128us) is the roofline.
  GroupNorm: fp32 bn_stats per channel; cross-partition group aggregation
    via block-diagonal f32r matmul; rsqrt as exp(-0.5*ln(var+eps)) so the
    whole kernel uses ONE activation table (no LoadActFuncSet switches);
    h = x*s + b' stored fp8e4 in paired c-tiles h2[cp] = [128, (j, s)].
    Batch-0 h is written by ACT Identity (idle pre-exp); batch-1 on DVE.
    Batch-1 GN is emitted inside batch-0's head-0 exp window so its DVE
    work never delays batch-0's q/k epilogues.
  QKV: q,k in [o, s] o-tile layout (head pair per tile at partition 0/64);
    q bias folded, k bias dropped (softmax-invariant).
    v^T per t-tile pair: v2[m] = [t128, (j, head, 66)], ones col at 64.
  Attention: logits^T[t,s] per (head,tt,sc) via one DoubleRow matmul;
    exp (no max subtraction: |logit| < ~3) -> fp8 P^T pairs pt2[m] =
    [t128, (j, s)]; AV accumulates 4 DoubleRow matmuls (M=65, ones row ->
    row 64 = softmax denominator Z); normalize via DVE reciprocal + gpsimd
    partition_broadcast + DVE multiply -> fp8 ao2[cp] = [128, (j, s)].
  Proj: DoubleRow over 2 c-pairs; epilogue is one scalar_tensor_tensor:
    out = (proj_psum + b_eff) + x, with x read from the resident xg tiles
    (b_eff = proj_b + proj_w @ b_v; valid because sum_t P/Z = 1 exactly).

Engine budget per core (modeled): ACT ~130us (exp-bound), DVE ~85us,
PE ~58us, POOL ~26us.
"""
import numpy as np
import ml_dtypes

import concourse.bass as bass
import concourse.bacc as bacc
import concourse.tile as tile
import concourse.mybir as mybir
from concourse import bass_utils

F32 = mybir.dt.float32
F32R = mybir.dt.float32r
F8 = mybir.dt.float8e4
I8 = mybir.dt.int8
AF = mybir.ActivationFunctionType
ALU = mybir.AluOpType
DR = mybir.MatmulPerfMode.DoubleRow

NCORES = 8
B, C, H, W = 16, 512, 32, 32
S = H * W                 # 1024
NH, HD = 8, 64
G, GS = 32, 16            # groups, channels per group
BLOC = B // NCORES        # 2
CT = C // 128             # 4 channel tiles
CP = CT // 2              # 2 channel-tile pairs
TT = S // 128             # 8 t tiles
TM = TT // 2              # 4 t-tile pairs
SC = S // 512             # 2 s chunks of 512
EPS = 1e-5

_NC_CACHE = {}


def f32r_round(x: np.ndarray) -> np.ndarray:
    """fp32 -> fp32r: round mantissa to 11 bits (round-half-away on bit 12)."""
    u = np.ascontiguousarray(x, dtype=np.float32).view(np.uint32)
    lsb = (u >> 12) & np.uint32(1)
    r = u + np.uint32(0x7FF) + lsb
    return (r & np.uint32(0xFFFFF000)).view(np.float32)


def build_program():
    # The greedy act-table picker takes the FIRST set containing each
    # function: Ln would pick the ln-only 'natural_log' set and every
    # following Exp would reload a different one (1.28us per reload, some
    # mid exp-stream). Blank 'natural_log' in the cached dict (shared
    # object, index positions preserved) so Ln resolves to
    # 'natural_log_exp_and_others' and the whole kernel runs off one table.
    if not getattr(bacc, "_nlx_patched", False):
        _orig_gat = bacc.get_activation_tables

        def _patched_gat(arch):
            tabs = dict(_orig_gat(arch))
            if "natural_log" in tabs:
                tabs["natural_log"] = set()
            return tabs

        bacc.get_activation_tables = _patched_gat
        bacc._nlx_patched = True

    nc = bacc.Bacc("TRN2", target_bir_lowering=False, debug=False)

    x_d = nc.dram_tensor("x", [BLOC, C, S], F32, kind="ExternalInput").ap()
    # [c_local, (cp 2, j 2, o 1024)]; o: 512 q channels (pre-scaled), 512 k
    wqk_d = nc.dram_tensor("wqk", [128, 2 * 2 * 1024], F8,
                           kind="ExternalInput").ap()
    # [c_local, (cp 2, j 2, o 512)]
    wv_d = nc.dram_tensor("wv", [128, 2 * 2 * 512], F8, kind="ExternalInput").ap()
    wp_d = nc.dram_tensor("wp", [128, 2 * 2 * 512], F8, kind="ExternalInput").ap()
    # packed small consts: [bq 4 | beff 4 | gamma 4 | beta 4 | gmat 128]
    smalls_d = nc.dram_tensor("smalls", [128, 144], F32R,
                              kind="ExternalInput").ap()
    out_d = nc.dram_tensor("out", [BLOC, C, S], F32, kind="ExternalOutput").ap()

    with tile.TileContext(nc) as tc:
        import contextlib
        with contextlib.ExitStack() as ctx:
            consts = ctx.enter_context(tc.tile_pool(name="consts", bufs=1))
            xg_pool = ctx.enter_context(tc.tile_pool(name="xg", bufs=2 * CT))
            small = ctx.enter_context(tc.tile_pool(name="small", bufs=4))
            h_pool = ctx.enter_context(tc.tile_pool(name="h", bufs=2 * CP))
            qk_pool = ctx.enter_context(tc.tile_pool(name="qk", bufs=16))
            v_pool = ctx.enter_context(tc.tile_pool(name="v", bufs=2 * TM))
            pt_pool = ctx.enter_context(tc.tile_pool(name="pt", bufs=4 * TM))
            ao_pool = ctx.enter_context(tc.tile_pool(name="ao", bufs=2 * CP))
            z_pool = ctx.enter_context(tc.tile_pool(name="z", bufs=2))
            res_pool = ctx.enter_context(tc.tile_pool(name="res", bufs=8))
            ps = ctx.enter_context(tc.tile_pool(name="ps", bufs=2, space="PSUM"))

            # ---- DMA order (one serial pipe, ~1.46us per x tile): small
            # gn constants, x b0, x b1, then weights (first QKV needs wqk
            # only at ~15us). xg tiles stay resident until the proj
            # residual reads them ----
            smalls_sb = consts.tile([128, 144], F32R, name="smalls")
            nc.sync.dma_start(out=smalls_sb, in_=smalls_d)
            bq_sb = smalls_sb[:, 0:4].bitcast(F32)
            beff_sb = smalls_sb[:, 4:8].bitcast(F32)
            gamma_sb = smalls_sb[:, 8:12].bitcast(F32)
            beta_sb = smalls_sb[:, 12:16].bitcast(F32)
            gmat_sb = smalls_sb[:, 16:144]
            eps_sb = consts.tile([128, 1], F32)
            nc.vector.memset(eps_sb, EPS)
            xg_tiles = {}
            for ci in range(CT):
                xg = xg_pool.tile([128, S], F32, tag="xg", name=f"xg0_{ci}")
                nc.sync.dma_start(out=xg, in_=x_d[0, ci * 128:(ci + 1) * 128, :])
                xg_tiles[(0, ci)] = xg
            wqk_sb = consts.tile([128, 2, 2, 1024], F8, name="wqk")
            nc.sync.dma_start(out=wqk_sb, in_=wqk_d)
            for ci in range(CT):
                xg = xg_pool.tile([128, S], F32, tag="xg", name=f"xg1_{ci}")
                nc.sync.dma_start(out=xg, in_=x_d[1, ci * 128:(ci + 1) * 128, :])
                xg_tiles[(1, ci)] = xg
            wv_sb = consts.tile([128, 2, 2, 512], F8, name="wv")
            nc.sync.dma_start(out=wv_sb, in_=wv_d)
            wp_sb = consts.tile([128, 2, 2, 512], F8, name="wp")
            nc.sync.dma_start(out=wp_sb, in_=wp_d)

            # ---- groupnorm; h stored fp8 in paired c-tiles ----
            h_tiles = {}   # (b, cp) -> [128, 2, S] fp8
            for b in range(BLOC):
                for cp in range(CP):
                    h_tiles[(b, cp)] = h_pool.tile(
                        [128, 2, S], F8, tag="h", name=f"h{b}_{cp}")

            def emit_gn(b):
                # per-ci bn stats/aggr, then ONE batched small-op chain over
                # all 4 ci columns (short DVE critical chain at startup)
                stats4 = small.tile([128, CT, 2, 6], F32, tag="stats4")
                mv4 = small.tile([128, CT, 2], F32, tag="mv4")
                for ci in range(CT):
                    xg2 = xg_tiles[(b, ci)].rearrange("p (n f) -> p n f", f=512)
                    nc.vector.bn_stats(out=stats4[:, ci, 0, :], in_=xg2[:, 0, :])
                    nc.vector.bn_stats(out=stats4[:, ci, 1, :], in_=xg2[:, 1, :])
                for ci in range(CT):
                    nc.vector.bn_aggr(out=mv4[:, ci, :], in_=stats4[:, ci, :, :])
                sin4 = small.tile([128, CT, 2], F32R, tag="sin4")
                nc.vector.tensor_copy(sin4[:, :, 0:1], mv4[:, :, 0:1])
                nc.vector.tensor_tensor(
                    out=sin4[:, :, 1:2], in0=mv4[:, :, 0:1], in1=mv4[:, :, 0:1],
                    op=ALU.mult)
                nc.vector.tensor_tensor(
                    out=sin4[:, :, 1:2], in0=sin4[:, :, 1:2].bitcast(F32),
                    in1=mv4[:, :, 1:2], op=ALU.add)
                gp4 = ps.tile([128, 2 * CT], F32, tag="mm")
                nc.tensor.matmul(gp4, gmat_sb,
                                 sin4.rearrange("p a b -> p (a b)"),
                                 start=True, stop=True)
                gg4 = small.tile([128, CT, 2], F32, tag="gg4")
                nc.vector.tensor_copy(gg4.rearrange("p a b -> p (a b)"), gp4)
                gv4 = small.tile([128, 5, CT], F32, tag="gv4")
                mean4 = gg4[:, :, 0:1].rearrange("p a b -> p (a b)")
                ex24 = gg4[:, :, 1:2].rearrange("p a b -> p (a b)")
                nc.vector.tensor_tensor(
                    out=gv4[:, 0, :], in0=mean4, in1=mean4, op=ALU.mult)
                nc.vector.tensor_tensor(
                    out=gv4[:, 0, :], in0=ex24, in1=gv4[:, 0, :], op=ALU.subtract)
                # rsqrt(var+eps) = exp(-0.5*ln(var+eps)): stays in the Exp
                # activation table (a Sqrt would force table reloads)
                nc.scalar.activation(gv4[:, 4, :], gv4[:, 0, :], AF.Ln,
                                     bias=eps_sb)
                nc.scalar.activation(gv4[:, 1, :], gv4[:, 4, :], AF.Exp,
                                     scale=-0.5)
                # tiny post-exp muls on Pool: keeps them out of the DVE
                # stream, which the tile scheduler may reorder behind
                # DMA-blocked bn_stats of the other batch
                nc.gpsimd.tensor_tensor(
                    out=gv4[:, 2, :], in0=gv4[:, 1, :], in1=gamma_sb[:, 0:CT],
                    op=ALU.mult)
                nc.gpsimd.tensor_tensor(
                    out=gv4[:, 3, :], in0=mean4, in1=gv4[:, 2, :], op=ALU.mult)
                nc.gpsimd.tensor_tensor(
                    out=gv4[:, 3, :], in0=beta_sb[:, 0:CT], in1=gv4[:, 3, :],
                    op=ALU.subtract)
                for ci in range(CT):
                    hdst = h_tiles[(b, ci // 2)][:, ci % 2, :]
                    if b == 0 and ci < 2:
                        # ACT is idle before the first exp; split h-writes
                        # ACT/DVE so neither engine serializes all four
                        nc.scalar.activation(
                            hdst, xg_tiles[(b, ci)], AF.Identity,
                            bias=gv4[:, 3, ci:ci + 1], scale=gv4[:, 2, ci:ci + 1])
                    else:
                        nc.vector.tensor_scalar(
                            out=hdst, in0=xg_tiles[(b, ci)],
                            scalar1=gv4[:, 2, ci:ci + 1],
                            scalar2=gv4[:, 3, ci:ci + 1],
                            op0=ALU.mult, op1=ALU.add)

            emit_gn(0)

            # ---- per batch: QKV -> attention -> proj ----
            qk_tiles = {}  # (b, ot) -> [128, S] fp8; ot 0-3 q, 4-7 k
            v_tiles = {}   # (b, m) -> [128, 2, NH, 66] fp8

            def emit_qk_tile(b, ot, k_on_act=False):
                qt = qk_pool.tile([128, S], F8, tag="qk", name=f"qk{b}_{ot}")
                for sc in range(SC):
                    pp = ps.tile([128, 512], F32, tag="mm", name="qkmm")
                    for cp in range(CP):
                        nc.tensor.matmul(
                            pp,
                            wqk_sb[:, cp, :, ot * 128:(ot + 1) * 128],
                            h_tiles[(b, cp)][:, :, sc * 512:(sc + 1) * 512],
                            start=(cp == 0), stop=(cp == CP - 1),
                            perf_mode=DR)
                    dst = qt[:, sc * 512:(sc + 1) * 512]
                    if ot < 4:
                        nc.vector.tensor_scalar(
                            out=dst, in0=pp, scalar1=bq_sb[:, ot:ot + 1],
                            scalar2=None, op0=ALU.add)
                    elif k_on_act:
                        # pre-exp ACT is idle: parallelize the first k
                        # epilogues with the q ones on DVE
                        nc.scalar.activation(dst, pp, AF.Identity)
                    else:
                        nc.vector.tensor_copy(dst, pp)
                qk_tiles[(b, ot)] = qt

            def emit_v_tiles(b):
                for m in range(TM):
                    vt = v_pool.tile([128, 2, NH, 66], F8, tag="v",
                                     name=f"v{b}_{m}")
                    for jt in range(2):         # t tile within pair
                        st = 2 * m + jt
                        pp = ps.tile([128, 512], F32, tag="mm", name="vmm")
                        for cp in range(CP):
                            nc.tensor.matmul(
                                pp,
                                h_tiles[(b, cp)][:, :, st * 128:(st + 1) * 128],
                                wv_sb[:, cp, :, :],
                                start=(cp == 0), stop=(cp == CP - 1),
                                perf_mode=DR)
                        nc.vector.tensor_copy(
                            vt[:, jt, :, 0:64],
                            pp.rearrange("p (h e) -> p h e", e=64))
                    nc.vector.memset(vt[:, :, :, 64:65], 1.0)
                    v_tiles[(b, m)] = vt

            ao_all = {}

            def emit_proj(b):
                for ot in range(CT):
                    for sc in range(SC):
                        pp = ps.tile([128, 512], F32, tag="mm")
                        for cp in range(CP):
                            nc.tensor.matmul(
                                pp,
                                wp_sb[:, cp, :, ot * 128:(ot + 1) * 128],
                                ao_all[b][cp][:, :, sc * 512:(sc + 1) * 512],
                                start=(cp == 0), stop=(cp == CP - 1),
                                perf_mode=DR)
                        ro = res_pool.tile([128, 512], F32, tag="ro")
                        nc.vector.scalar_tensor_tensor(
                            out=ro, in0=pp, scalar=beff_sb[:, ot:ot + 1],
                            in1=xg_tiles[(b, ot)][:, sc * 512:(sc + 1) * 512],
                            op0=ALU.add, op1=ALU.add)
                        # tail outputs split across the SP and ACT queues
                        eng = nc.sync if (b == 0 or (ot * SC + sc) % 2 == 0) \
                            else nc.scalar
                        eng.dma_start(
                            out=out_d[b, ot * 128:(ot + 1) * 128,
                                      sc * 512:(sc + 1) * 512],
                            in_=ro)

            # exp load-balancing: DVE computes some tiles with a one-op
            # Schraudolph exp directly in fp8e4m3 bits:
            #   int8 = round(logit * 8/ln2 + SCH_B)  ==  bits of exp(logit)
            # (piecewise-linear 2^x; error ~ fp8 quantization, verified
            # numerically to match the table-exp path's accuracy)
            SCH_A = 8.0 / float(np.log(2.0))
            SCH_B = 55.75
            DVE_EXP = {(0, 0): 0, (0, 1): 0, (0, 2): 0, (1, 0): 0}
            DVE_TILES = {0: set(), 1: {(0, 1)}, 2: {(0, 1), (2, 1)},
                         3: {(0, 1), (1, 1), (2, 1)}}
            DVE_EXP_DEFAULT = 1

            def emit_qk_exp(b, h):
                """logits + exp for head h -> pt tiles [128, 2, S] fp8."""
                p0 = 64 * (h % 2)
                kt = qk_tiles[(b, 4 + h // 2)]
                qt = qk_tiles[(b, h // 2)]
                dve_tiles = DVE_TILES[DVE_EXP.get((b, h), DVE_EXP_DEFAULT)]
                pts = []
                for m in range(TM):
                    pt = pt_pool.tile([128, 2, S], F8, tag="pt",
                                      name=f"pt{b}_{h}_{m}")
                    for jt in range(2):
                        t0 = (2 * m + jt) * 128
                        lg = ps.tile([128, 1024], F32, tag="qk", name="lg")
                        for sc in range(SC):
                            nc.tensor.matmul(
                                lg[:, sc * 512:(sc + 1) * 512],
                                kt[p0:p0 + 64, t0:t0 + 128]
                                .unsqueeze(1).broadcast_to([64, 2, 128]),
                                qt[p0:p0 + 64, sc * 512:(sc + 1) * 512]
                                .unsqueeze(1).broadcast_to([64, 2, 512]),
                                start=True, stop=True, perf_mode=DR)
                        if (m, jt) in dve_tiles:
                            nc.vector.tensor_scalar(
                                out=pt[:, jt, :].bitcast(I8), in0=lg,
                                scalar1=SCH_A, scalar2=SCH_B,
                                op0=ALU.mult, op1=ALU.add)
                        else:
                            nc.scalar.activation(pt[:, jt, :], lg, AF.Exp)
                    pts.append(pt)
                return pts

            def emit_av(b, h, pts):
                cp, j, r0 = h // 4, (h // 2) % 2, (h % 2) * 64
                for sc in range(SC):
                    av = ps.tile([65, 512], F32, tag="av")
                    for m in range(TM):
                        nc.tensor.matmul(
                            av,
                            v_tiles[(b, m)][:, :, h, 0:65],
                            pts[m][:, :, sc * 512:(sc + 1) * 512],
                            start=(m == 0), stop=(m == TM - 1),
                            perf_mode=DR)
                    zr = z_pool.tile([1, 512], F32, tag="zr")
                    nc.vector.reciprocal(out=zr, in_=av[64:65, :])
                    zb = z_pool.tile([64, 512], F32, tag="zb")
                    nc.gpsimd.partition_broadcast(zb, zr, channels=64)
                    nc.vector.tensor_tensor(
                        out=ao_all[b][cp][r0:r0 + 64, j,
                                          sc * 512:(sc + 1) * 512],
                        in0=av[0:64, :], in1=zb, op=ALU.mult)

            for b in range(BLOC):
                for cp in range(CP):
                    ao_all.setdefault(b, {})[cp] = ao_pool.tile(
                        [128, 2, S], F8, tag="ao", name=f"ao{b}_{cp}")

            # PE warm-up: keep PE continuously busy through the DMA/GN
            # startup so the first QKV matmuls run at full clock (any PE
            # idle resets the pstate ramp)

            emit_qk_tile(0, 0, k_on_act=True)
            emit_qk_tile(0, 4, k_on_act=True)
            emit_v_tiles(0)
            # (b1 GN + prep are hoisted into b0's exp windows below)

            prev = None
            for b, h in [(bb, hh) for bb in range(BLOC) for hh in range(NH)]:
                cur = emit_qk_exp(b, h)
                if b == 0 and h == 0:
                    # nearest exp deadline first: b0's next pair
                    emit_qk_tile(0, 1)
                    emit_qk_tile(0, 5)
                if b == 0 and h == 1:
                    # b1 GN here (not h0): its gmat matmul waits on b1's
                    # DVE stats; at h0 that stall would block QK(h1) on
                    # the in-order PE queue
                    emit_qk_tile(0, 2)
                    emit_qk_tile(0, 6)
                    emit_gn(1)
                if b == 0 and h == 2:
                    emit_qk_tile(0, 3)
                    emit_qk_tile(0, 7)
                    emit_qk_tile(1, 0)
                    emit_qk_tile(1, 4)
                    emit_v_tiles(1)
                if b == 0 and h == 3:
                    for pr in range(1, 4):
                        emit_qk_tile(1, pr)
                        emit_qk_tile(1, pr + 4)
                if prev is not None:
                    emit_av(*prev)
                if b == 1 and h == 0:
                    # b0's proj here: after AV(b0,h7) wrote the last ao rows,
                    # and after QK(b1,h0) so b1's first exps aren't delayed
                    emit_proj(0)
                prev = (b, h, cur)
            emit_av(*prev)

            emit_proj(1)

    nc.compile()
    return nc


def prep_weights(norm_w, norm_b, qkv_w, qkv_b, proj_w, proj_b):
    """Host-side constant preprocessing."""
    f8 = ml_dtypes.float8_e4m3
    # extra 1/2: the QK DoubleRow broadcast pair sums the same product twice
    scale = 1.0 / (2.0 * np.sqrt(HD))
    wq = qkv_w[0:C] * scale
    wk = qkv_w[C:2 * C]
    wv = qkv_w[2 * C:3 * C]
    bq = qkv_b[0:C] * scale
    bv = qkv_b[2 * C:3 * C]

    # wqk2: [c_local, cp, j, o(1024)]; o 0-511 = q channels, 512-1023 = k
    wqk_cat = np.concatenate([wq, wk], axis=0)       # [1024, 512]
    wqk2 = np.zeros((128, 2, 2, 1024), dtype=f8)
    wv2 = np.zeros((128, 2, 2, 512), dtype=f8)
    wp2 = np.zeros((128, 2, 2, 512), dtype=f8)
    for cp in range(CP):
        for j in range(2):
            c0 = (2 * cp + j) * 128
            wqk2[:, cp, j, :] = wqk_cat[:, c0:c0 + 128].T.astype(f8)
            wv2[:, cp, j, :] = wv[:, c0:c0 + 128].T.astype(f8)
            wp2[:, cp, j, :] = proj_w[:, c0:c0 + 128].T.astype(f8)

    bq2 = bq.reshape(CT, 128).T.astype(np.float32)
    beff = (proj_b + proj_w @ bv).reshape(CT, 128).T.astype(np.float32)
    gamma = norm_w.reshape(CT, 128).T.astype(np.float32)
    beta = norm_b.reshape(CT, 128).T.astype(np.float32)
    gmat = np.zeros((128, 128), dtype=np.float32)
    for g in range(128 // GS):
        gmat[g * GS:(g + 1) * GS, g * GS:(g + 1) * GS] = 1.0 / GS
    gmat = f32r_round(gmat)
    smalls = np.concatenate([bq2, beff, gamma, beta, gmat], axis=1)
    return dict(wqk=wqk2.reshape(128, -1), wv=wv2.reshape(128, -1),
                wp=wp2.reshape(128, -1),
                smalls=np.ascontiguousarray(smalls.astype(np.float32)))


def kernel(x, norm_w, norm_b, qkv_w, qkv_b, proj_w, proj_b, _trace=False):
    x = np.ascontiguousarray(np.asarray(x, dtype=np.float32))
    consts = prep_weights(
        np.asarray(norm_w, np.float32), np.asarray(norm_b, np.float32),
        np.asarray(qkv_w, np.float32), np.asarray(qkv_b, np.float32),
        np.asarray(proj_w, np.float32), np.asarray(proj_b, np.float32))

    if "nc" not in _NC_CACHE:
        _NC_CACHE["nc"] = build_program()
    nc = _NC_CACHE["nc"]

    xr = x.reshape(B, C, S)
    in_maps = []
    for core in range(NCORES):
        m = dict(consts)
        m["x"] = np.ascontiguousarray(xr[core * BLOC:(core + 1) * BLOC])
        in_maps.append(m)

    res = bass_utils.run_bass_kernel_spmd(
        nc, in_maps, core_ids=list(range(NCORES)), trace=False)

    out = np.empty((B, C, S), dtype=np.float32)
    for core in range(NCORES):
        out[core * BLOC:(core + 1) * BLOC] = res.results[core]["out"]
    kernel.last_results = res
    return out.reshape(B, C, H, W)
